# revision 1
# baseline (speedup 1.0000x reference)
"""Trainium2 Bass kernel for 2-layer GCN (nn_GCN_39848706573686).

Node-sharded across 8 NeuronCores (12500 nodes/core + pad). Three SPMD
launches:
  L1: g = deg^-1/2 * (x @ W1) per-core shard          (TensorE + DVE)
  L2: conv1 padded-ELL segment reduce + bias/relu/W2  (DVE/ACT + TensorE)
  L3: conv2 padded-ELL segment reduce + bias          (DVE)
The host performs only integer routing: edge bucketing by destination,
degree counting, ELL slot index construction, and the halo-exchange row
replication between launches (device collectives / indirect DMA are not
available under this axon terminal). All floating-point arithmetic runs
on the NeuronCores.
"""
import os
import sys
import types
import numpy as np

# --- environment bootstrap (self-contained copy of bassboot logic) -----
for _p in ("/opt/trn_rl_repo", "/root/patched"):
    if _p not in sys.path and os.path.isdir(_p):
        sys.path.insert(0, _p)

from concourse import bass, bacc, mybir, tile  # noqa: E402
from concourse import bass_utils  # noqa: E402


def _install_ntff_hook():
    if "antenv.axon_hooks" not in sys.modules:
        mod = types.ModuleType("antenv.axon_hooks")
        _h = {}
        mod.set_axon_ntff_profile_hook = lambda h: _h.__setitem__("h", h)
        mod.get_axon_ntff_profile_hook = lambda: _h.get("h")
        sys.modules["antenv.axon_hooks"] = mod
        try:
            import antenv
            antenv.axon_hooks = mod
        except ImportError:
            pass
    mod = sys.modules["antenv.axon_hooks"]
    if mod.get_axon_ntff_profile_hook() is None:
        try:
            from trn_agent_boot.trn_boot import _ntff_profile_via_ctypes
            hook = _ntff_profile_via_ctypes("/opt/axon/libaxon_pjrt.so")
            if hook is not None:
                mod.set_axon_ntff_profile_hook(hook)
        except Exception:
            pass
    bass_utils.upload_artifacts = lambda tmpdir: str(tmpdir)


_install_ntff_hook()

# --- problem constants -------------------------------------------------
N, E, F, H = 100000, 3200000, 128, 16
NC = 8
SH = 12500                  # real nodes per core
SHP = 12544                 # padded rows per core (= 98 * 128)
NB = 98                     # node blocks of 128 per core
CHUNK_COLS = 12288           # max slot columns per DMA chunk (per partition)

FT = mybir.dt.float32
BF = mybir.dt.bfloat16
USE_BF16 = os.environ.get("GCN_BF16", "1") == "1"
ST = BF if USE_BF16 else FT

_cached = {}

# Track total device time across launches for test harness
last_exec_ns = {}


def _build_l1():
    """g^T = (dis * (x @ W1))^T computed as W1^T-stationary matmuls over
    x^T chunks; output [16, SHP] (host transposes)."""
    CH1 = 448
    NCH1 = SHP // CH1          # 28 chunks
    NPIECE = 4
    PIECE = SHP // NPIECE      # 3136 cols per input DMA piece
    nc = bacc.Bacc("TRN2", target_bir_lowering=False, debug=False,
                   num_devices=NC)
    xT = nc.dram_tensor("xT", [128, SHP], ST, kind="ExternalInput").ap()
    w1 = nc.dram_tensor("w1", [128, H], ST, kind="ExternalInput").ap()
    dis16 = nc.dram_tensor("dis16", [16, SHP], FT, kind="ExternalInput").ap()
    g = nc.dram_tensor("g", [16, SHP], FT, kind="ExternalOutput").ap()
    with tile.TileContext(nc) as tc:
        with tc.tile_pool(name="sb", bufs=NPIECE) as sb, \
             tc.tile_pool(name="cst", bufs=1) as cst, \
             tc.tile_pool(name="ps", bufs=8, space="PSUM") as ps:
            w1_t = cst.tile([128, H], ST)
            nc.sync.dma_start(out=w1_t[:], in_=w1[:])
            d16_t = cst.tile([16, SHP], FT)
            nc.sync.dma_start(out=d16_t[:], in_=dis16[:])
            g_t = cst.tile([16, SHP], FT)
            pieces = []
            for pc in range(NPIECE):
                xt_p = sb.tile([128, PIECE], ST, tag="xtp")
                nc.sync.dma_start(
                    out=xt_p[:], in_=xT[:, pc * PIECE:(pc + 1) * PIECE])
                pieces.append(xt_p)
            per_piece = PIECE // CH1
            for t in range(NCH1):
                pc, off = t // per_piece, (t % per_piece) * CH1
                p = ps.tile([16, CH1], FT, space="PSUM")
                nc.tensor.matmul(out=p[:], lhsT=w1_t[:],
                                 rhs=pieces[pc][:, off:off + CH1],
                                 start=True, stop=True)
                c0 = t * CH1
                nc.vector.tensor_tensor(
                    out=g_t[:, c0:c0 + CH1], in0=p[:],
                    in1=d16_t[:, c0:c0 + CH1], op=mybir.AluOpType.mult)
            nc.sync.dma_start(out=g[:], in_=g_t[:])
    nc.compile()
    return nc


def _plan_chunks(caps, d):
    """Split the 98 blocks into DMA chunks and equal-cap reduce segments.

    Returns (cols_total, chunks) where chunks = list of
    (col_off, col_len, [(cap, b_start, nb, seg_off), ...]).
    Column unit = slot elements per partition (d * cap per block).
    """
    chunks = []
    cur = []            # list of (b, cap)
    cur_cols = 0
    col_off = 0
    offs = np.concatenate([[0], np.cumsum([d * c for c in caps])]).astype(int)
    bi = 0
    while bi < NB:
        c = d * caps[bi]
        if cur and cur_cols + c > CHUNK_COLS:
            chunks.append((col_off, cur_cols, cur))
            col_off += cur_cols
            cur, cur_cols = [], 0
        cur.append((bi, caps[bi]))
        cur_cols += c
        bi += 1
    if cur:
        chunks.append((col_off, cur_cols, cur))
    out = []
    for (coff, clen, blist) in chunks:
        segs = []
        j = 0
        seg_off = 0
        while j < len(blist):
            b0, cap = blist[j]
            nb = 1
            while j + nb < len(blist) and blist[j + nb][1] == cap:
                nb += 1
            segs.append((cap, b0, nb, seg_off))
            seg_off += nb * d * cap
            j += nb
        out.append((coff, clen, segs))
    return int(offs[-1]), out


def _reduce_caps(nc, sb, res_t, slots, caps, d, dis_t=None):
    """Stream slot columns, reduce each block's cap slots; optionally fuse
    the per-node dis multiply (L2) right after each segment's reduce."""
    cols, chunks = _plan_chunks(caps, d)
    maxlen = max(cl for _, cl, _ in chunks)
    for (coff, clen, segs) in chunks:
        st = sb.tile([128, maxlen], ST, tag="slotbuf")
        nc.sync.dma_start(out=st[:, :clen], in_=slots[:, coff:coff + clen])
        for (cap, b0, nb, soff) in segs:
            src = st[:, soff:soff + nb * d * cap]
            if d > 1:
                src = src.rearrange("p (b c w) -> p b c w", b=nb, c=d, w=cap)
                out_ap = res_t[:, b0 * d:(b0 + nb) * d].rearrange(
                    "p (b c) -> p b c", b=nb, c=d)
            else:
                src = src.rearrange("p (b w) -> p b w", b=nb, w=cap)
                out_ap = res_t[:, b0:b0 + nb]
            nc.vector.tensor_reduce(out=out_ap, in_=src,
                                    axis=mybir.AxisListType.X,
                                    op=mybir.AluOpType.add)
            if dis_t is not None:
                for b in range(b0, b0 + nb):
                    nc.vector.tensor_scalar(
                        out=res_t[:, d * b:d * (b + 1)],
                        in0=res_t[:, d * b:d * (b + 1)],
                        scalar1=dis_t[:, b:b + 1], scalar2=None,
                        op0=mybir.AluOpType.mult)


def _build_l2(caps):
    cols, _ = _plan_chunks(caps, H)
    nc = bacc.Bacc("TRN2", target_bir_lowering=False, debug=False,
                   num_devices=NC)
    slots = nc.dram_tensor("slots", [128, cols], ST,
                           kind="ExternalInput").ap()
    dis = nc.dram_tensor("dis", [128, NB], FT, kind="ExternalInput").ap()
    b1s = nc.dram_tensor("b1s", [128, H], FT, kind="ExternalInput").ap()
    w2s = nc.dram_tensor("w2s", [128, H], FT, kind="ExternalInput").ap()
    g2 = nc.dram_tensor("g2", [128, NB], FT, kind="ExternalOutput").ap()
    with tile.TileContext(nc) as tc:
        with tc.tile_pool(name="sb", bufs=3) as sb, \
             tc.tile_pool(name="cst", bufs=1) as cst:
            res_t = cst.tile([128, NB * H], FT)
            dis_t = cst.tile([128, NB], FT)
            nc.sync.dma_start(out=dis_t[:], in_=dis[:])
            b1s_t = cst.tile([128, H], FT)
            nc.sync.dma_start(out=b1s_t[:], in_=b1s[:])
            w2s_t = cst.tile([128, H], FT)
            nc.sync.dma_start(out=w2s_t[:], in_=w2s[:])
            b1_t = cst.tile([128, NB * H], FT)
            nc.vector.tensor_copy(
                out=b1_t[:].rearrange("p (b c) -> p b c", b=NB, c=H),
                in_=b1s_t[:].unsqueeze(1).to_broadcast([128, NB, H]))
            w2_t = cst.tile([128, NB * H], FT)
            nc.vector.tensor_copy(
                out=w2_t[:].rearrange("p (b c) -> p b c", b=NB, c=H),
                in_=w2s_t[:].unsqueeze(1).to_broadcast([128, NB, H]))
            disr = cst.tile([128, NB * H], FT)
            nc.vector.tensor_copy(
                out=disr[:].rearrange("p (b c) -> p b c", b=NB, c=H),
                in_=dis_t[:].unsqueeze(2).to_broadcast([128, NB, H]))
            _reduce_caps(nc, sb, res_t, slots, caps, H)
            # out1 = dis*agg + b1 ; relu ; * w2 ; sum H ; * dis
            nc.vector.tensor_tensor(out=res_t[:], in0=res_t[:], in1=disr[:],
                                    op=mybir.AluOpType.mult)
            nc.vector.tensor_tensor(out=res_t[:], in0=res_t[:], in1=b1_t[:],
                                    op=mybir.AluOpType.add)
            nc.vector.tensor_scalar(out=res_t[:], in0=res_t[:], scalar1=0.0,
                                    scalar2=None, op0=mybir.AluOpType.max)
            nc.vector.tensor_tensor(out=res_t[:], in0=res_t[:], in1=w2_t[:],
                                    op=mybir.AluOpType.mult)
            g2_t = cst.tile([128, NB], FT)
            nc.vector.tensor_reduce(
                out=g2_t[:],
                in_=res_t[:].rearrange("p (b c) -> p b c", b=NB, c=H),
                axis=mybir.AxisListType.X, op=mybir.AluOpType.add)
            nc.vector.tensor_tensor(out=g2_t[:], in0=g2_t[:], in1=dis_t[:],
                                    op=mybir.AluOpType.mult)
            nc.sync.dma_start(out=g2[:], in_=g2_t[:])
    nc.compile()
    return nc


def _build_l3(caps):
    cols, _ = _plan_chunks(caps, 1)
    nc = bacc.Bacc("TRN2", target_bir_lowering=False, debug=False,
                   num_devices=NC)
    slots = nc.dram_tensor("slots", [128, cols], ST,
                           kind="ExternalInput").ap()
    dis = nc.dram_tensor("dis", [128, NB], FT, kind="ExternalInput").ap()
    b2 = nc.dram_tensor("b2", [128, NB], FT, kind="ExternalInput").ap()
    out = nc.dram_tensor("out", [128, NB], FT, kind="ExternalOutput").ap()
    with tile.TileContext(nc) as tc:
        with tc.tile_pool(name="sb", bufs=3) as sb, \
             tc.tile_pool(name="cst", bufs=1) as cst:
            res_t = cst.tile([128, NB], FT)
            dis_t = cst.tile([128, NB], FT)
            nc.sync.dma_start(out=dis_t[:], in_=dis[:])
            b2_t = cst.tile([128, NB], FT)
            nc.sync.dma_start(out=b2_t[:], in_=b2[:])
            _reduce_caps(nc, sb, res_t, slots, caps, 1)
            nc.vector.tensor_tensor(out=res_t[:], in0=res_t[:], in1=dis_t[:],
                                    op=mybir.AluOpType.mult)
            nc.vector.tensor_tensor(out=res_t[:], in0=res_t[:], in1=b2_t[:],
                                    op=mybir.AluOpType.add)
            nc.sync.dma_start(out=out[:], in_=res_t[:])
    nc.compile()
    return nc


def _run(nc, in_maps, label):
    trace = os.environ.get("GCN_TRACE", "0") == "1"
    res = bass_utils.run_bass_kernel_spmd(nc, in_maps,
                                          core_ids=list(range(NC)),
                                          trace=trace)
    if res.exec_time_ns is not None:
        last_exec_ns[label] = res.exec_time_ns
    return res.results


def kernel(x, edge_index, W1, b1, W2, b2):
    x = np.asarray(x, np.float32)
    edge_index = np.asarray(edge_index, np.int32)
    W1 = np.asarray(W1, np.float32)
    b1 = np.asarray(b1, np.float32)
    W2 = np.asarray(W2, np.float32)
    b2 = np.asarray(b2, np.float32)

    # ---- host routing (integer index work only) ----
    loop = np.arange(N, dtype=np.int64)
    src = np.concatenate([edge_index[0].astype(np.int64), loop])
    dst = np.concatenate([edge_index[1].astype(np.int64), loop])
    deg = np.bincount(dst, minlength=N).astype(np.int64)
    order = np.argsort(dst, kind="stable")
    src_s, dst_s = src[order], dst[order]
    core_start = np.searchsorted(dst_s, np.arange(0, N + 1, SH))

    # per-core degree-sorted row assignment + per-block slot caps
    pi = []           # pi[c][r] = global node id at row r (-1 = pad)
    caps_core = np.zeros((NC, NB), np.int64)
    for c in range(NC):
        d_loc = np.zeros(SHP, np.int64)
        d_loc[:SH] = deg[c * SH:(c + 1) * SH]
        ids = np.full(SHP, -1, np.int64)
        ids[:SH] = np.arange(c * SH, (c + 1) * SH)
        o = np.argsort(d_loc, kind="stable")
        rows = ids[o]
        pi.append(rows)
        dsorted = d_loc[o]
        caps_core[c] = np.maximum(
            4, ((dsorted.reshape(NB, 128).max(axis=1) + 3) // 4) * 4)
    caps = tuple(int(v) for v in caps_core.max(axis=0))
    offs16 = np.concatenate([[0], np.cumsum([H * cc for cc in caps])]).astype(np.int64)
    offs1 = np.concatenate([[0], np.cumsum(caps)]).astype(np.int64)
    COLS16, COLS1 = int(offs16[-1]), int(offs1[-1])

    dis_full = np.where(deg > 0, 1.0 / np.sqrt(deg.astype(np.float64)),
                        0.0).astype(np.float32)

    # ---- L1: g = dis * (x @ W1) on device ----
    import ml_dtypes
    SDT1 = ml_dtypes.bfloat16 if USE_BF16 else np.float32
    l1 = _cached.get("l1") or _cached.setdefault("l1", _build_l1())
    in_maps1 = []
    for c in range(NC):
        xs = np.zeros((SHP, F), np.float32)
        xs[:SH] = x[c * SH:(c + 1) * SH]
        dis_sh = np.zeros(SHP, np.float32)
        dis_sh[:SH] = dis_full[c * SH:(c + 1) * SH]
        in_maps1.append({"xT": np.ascontiguousarray(xs.T).astype(SDT1),
                         "w1": W1.astype(SDT1),
                         "dis16": np.ascontiguousarray(
                             np.broadcast_to(dis_sh[None, :], (16, SHP)))})
    res1 = _run(l1, in_maps1, "l1")
    g_full = np.zeros((N, H), np.float32)
    for c in range(NC):
        g_full[c * SH:(c + 1) * SH] = res1[c]["g"][:, :SH].T

    # ---- per-core slot coordinates (host, reused for L2/L3) ----
    coords = []       # (p_e, col0_2, cap_e, col_3, srcs_e)
    dis_dev = []
    sdt = np.dtype("bfloat16") if False else None
    import ml_dtypes
    SDT = ml_dtypes.bfloat16 if USE_BF16 else np.float32
    for c in range(NC):
        rows = pi[c]
        r = np.arange(SHP)
        valid = rows >= 0
        dis_t = np.zeros((128, NB), np.float32)
        dis_t[r % 128, r // 128] = np.where(
            valid, dis_full[np.where(valid, rows, 0)], 0.0)
        dis_dev.append(dis_t)
        rr = r[valid]
        nodes_r = rows[valid]
        st = core_start[c] + np.searchsorted(
            dst_s[core_start[c]:core_start[c + 1]],
            nodes_r)
        cnt = deg[nodes_r]
        rep_r = np.repeat(rr, cnt)
        w_e = np.arange(len(rep_r)) - np.repeat(np.cumsum(cnt) - cnt, cnt)
        srcs_e = src_s[np.repeat(st, cnt) + w_e]
        b_e = rep_r // 128
        p_e = rep_r % 128
        cap_e = np.asarray(caps)[b_e]
        col0_2 = offs16[b_e] + w_e
        col_3 = offs1[b_e] + w_e
        coords.append((p_e, col0_2, cap_e, col_3, srcs_e))

    # ---- L2: conv1 reduce + relu + W2 on device ----
    key2 = ("l2", caps)
    l2 = _cached.get(key2) or _cached.setdefault(key2, _build_l2(caps))
    b1_rep = np.tile(b1[None, :], (128, 1)).astype(np.float32)
    w2_rep = np.tile(W2[:, 0][None, :], (128, 1)).astype(np.float32)
    in_maps2 = []
    for c in range(NC):
        p_e, col0_2, cap_e, _, srcs_e = coords[c]
        sl = np.zeros((128, COLS16), SDT)
        gv = g_full[srcs_e]          # [E_c, H] f32
        gv = gv.astype(SDT)
        for ch in range(H):
            sl[p_e, col0_2 + ch * cap_e] = gv[:, ch]
        in_maps2.append({"slots": sl, "dis": dis_dev[c],
                         "b1s": b1_rep, "w2s": w2_rep})
    res2 = _run(l2, in_maps2, "l2")
    g2_full = np.zeros(N, np.float32)
    for c in range(NC):
        g2c = res2[c]["g2"]
        rows = pi[c]
        r = np.arange(SHP)
        valid = rows >= 0
        g2_full[rows[valid]] = g2c[(r % 128)[valid], (r // 128)[valid]]

    # ---- L3: conv2 reduce on device ----
    key3 = ("l3", caps)
    l3 = _cached.get(key3) or _cached.setdefault(key3, _build_l3(caps))
    in_maps3 = []
    b2_rep = np.full((128, NB), float(b2[0]), np.float32)
    for c in range(NC):
        p_e, _, _, col_3, srcs_e = coords[c]
        sl = np.zeros((128, COLS1), SDT)
        sl[p_e, col_3] = g2_full[srcs_e].astype(SDT)
        in_maps3.append({"slots": sl, "dis": dis_dev[c], "b2": b2_rep})
    res3 = _run(l3, in_maps3, "l3")
    out = np.zeros((N, 1), np.float32)
    for c in range(NC):
        oc = res3[c]["out"]
        rows = pi[c]
        r = np.arange(SHP)
        valid = rows >= 0
        out[rows[valid], 0] = oc[(r % 128)[valid], (r // 128)[valid]]
    return out



# revision 4
# speedup vs baseline: 1.4879x; 1.4879x over previous
"""Trainium2 Bass kernel for 2-layer GCN (nn_GCN_39848706573686).

Node-sharded across 8 NeuronCores (12500 nodes/core + pad). Three SPMD
launches (host does integer routing between them; all FP math on device):
  L1: g1 = dis * (x @ W1), node-on-partition layout      (TensorE + ACT + DVE)
  L2: conv1 padded-ELL segment reduce via bf16 tree-adds
      + bias/relu/W2 epilogue                            (DVE)
  L3: conv2 padded-ELL segment reduce + bias             (DVE)

The ELL slot arrays are slot-major per equal-cap segment: layout
[128 part, cap, nodes*d] so the segment reduce is a log2(cap) chain of
full-slab in-place tensor_tensor adds, which run in the DVE 2x bf16 perf
mode (vs tensor_reduce which is capped at 1x).
"""
import os
import sys
import types
import numpy as np

# --- environment bootstrap (self-contained copy of bassboot logic) -----
for _p in ("/opt/trn_rl_repo", "/root/patched"):
    if _p not in sys.path and os.path.isdir(_p):
        sys.path.insert(0, _p)

from concourse import bass, bacc, mybir, tile  # noqa: E402
from concourse import bass_utils  # noqa: E402


def _install_ntff_hook():
    if "antenv.axon_hooks" not in sys.modules:
        mod = types.ModuleType("antenv.axon_hooks")
        _h = {}
        mod.set_axon_ntff_profile_hook = lambda h: _h.__setitem__("h", h)
        mod.get_axon_ntff_profile_hook = lambda: _h.get("h")
        sys.modules["antenv.axon_hooks"] = mod
        try:
            import antenv
            antenv.axon_hooks = mod
        except ImportError:
            pass
    mod = sys.modules["antenv.axon_hooks"]
    if mod.get_axon_ntff_profile_hook() is None:
        try:
            from trn_agent_boot.trn_boot import _ntff_profile_via_ctypes
            hook = _ntff_profile_via_ctypes("/opt/axon/libaxon_pjrt.so")
            if hook is not None:
                mod.set_axon_ntff_profile_hook(hook)
        except Exception:
            pass
    bass_utils.upload_artifacts = lambda tmpdir: str(tmpdir)


_install_ntff_hook()

# --- problem constants -------------------------------------------------
N, E, F, H = 100000, 3200000, 128, 16
NC = 8
SH = 12500                  # real nodes per core
SHP = 12544                 # padded rows per core (= 98 * 128)
NB = 98                     # node blocks of 128 per core
CAP_R = 4                   # cap rounding
CH_COLS = 24576             # max slot columns per DMA chunk (per partition)

FT = mybir.dt.float32
BF = mybir.dt.bfloat16

_cached = {}

# Track total device time across launches for test harness
last_exec_ns = {}


# ---------------------------------------------------------------------
# plan: segments of equal cap, packed into DMA chunks
# ---------------------------------------------------------------------
def _plan(caps, d):
    """Segments = runs of equal cap (split to fit chunks).

    Returns (cols, chunks, colbase, stride) where
      chunks  = [(coff, clen, [(cap, b0, nb, soff), ...]), ...]
      colbase[b], stride[b]: edge (b, w, ch) -> col = colbase[b] + w*stride[b] + ch
    column unit = slot elements per partition; segment layout is
    [cap, nb*d] slot-major inside its column range.
    """
    runs = []
    b = 0
    while b < NB:
        b2 = b
        while b2 < NB and caps[b2] == caps[b]:
            b2 += 1
        runs.append((caps[b], b, b2 - b))
        b = b2
    chunks = []
    cur_segs, cur_cols, coff = [], 0, 0
    colbase = np.zeros(NB, np.int64)
    stride = np.zeros(NB, np.int64)

    def close():
        nonlocal cur_segs, cur_cols, coff
        if cur_segs:
            chunks.append((coff, cur_cols, cur_segs))
            coff += cur_cols
            cur_segs, cur_cols = [], 0

    for (cap, b0, nb) in runs:
        while nb > 0:
            fit = (CH_COLS - cur_cols) // (d * cap)
            if fit <= 0:
                close()
                fit = CH_COLS // (d * cap)
            take = min(nb, fit)
            soff = cur_cols
            cur_segs.append((cap, b0, take, soff))
            for j in range(take):
                colbase[b0 + j] = coff + soff + j * d
                stride[b0 + j] = take * d
            cur_cols += take * d * cap
            b0 += take
            nb -= take
    close()
    return int(coff), chunks, colbase, stride


# ---------------------------------------------------------------------
# device builders
# ---------------------------------------------------------------------
def _build_l1():
    """g1 = disrep * (x @ W1) in [128 nodes, NB*16] layout."""
    # whole-block DMA pieces of xT
    PIECES = [25, 25, 24, 24]           # blocks per piece
    nc = bacc.Bacc("TRN2", target_bir_lowering=False, debug=False,
                   num_devices=NC)
    xT = nc.dram_tensor("xT", [128, SHP], BF, kind="ExternalInput").ap()
    w1 = nc.dram_tensor("w1", [128, H], BF, kind="ExternalInput").ap()
    disrep = nc.dram_tensor("disrep", [128, NB * H], BF,
                            kind="ExternalInput").ap()
    g1 = nc.dram_tensor("g1", [128, NB * H], BF, kind="ExternalOutput").ap()
    with tile.TileContext(nc) as tc:
        with tc.tile_pool(name="sb", bufs=len(PIECES)) as sb, \
             tc.tile_pool(name="cst", bufs=1) as cst, \
             tc.tile_pool(name="ps", bufs=1, space="PSUM") as ps:
            w1_t = cst.tile([128, H], BF)
            nc.scalar.dma_start(out=w1_t[:], in_=w1[:])
            disrep_t = cst.tile([128, NB * H], BF)
            nc.scalar.dma_start(out=disrep_t[:], in_=disrep[:])
            g_sb = cst.tile([128, NB * H], BF)
            pieces = []
            off = 0
            for nb_p in PIECES:
                xt_p = sb.tile([128, 3200], BF, tag="xtp")
                nc.sync.dma_start(
                    out=xt_p[:, :nb_p * 128],
                    in_=xT[:, off * 128:(off + nb_p) * 128])
                pieces.append((xt_p, off))
                off += nb_p
            psts = [ps.tile([128, 512], FT, space="PSUM", name=f"pst{i}")
                    for i in range(4)]
            pc = 0
            for t in range(NB):
                while t >= pieces[pc][1] + PIECES[pc]:
                    pc += 1
                xt_p, poff = pieces[pc]
                loc = t - poff
                pst = psts[t // 32]
                nc.tensor.matmul(out=pst[:, (t % 32) * H:(t % 32 + 1) * H],
                                 lhsT=xt_p[:, loc * 128:(loc + 1) * 128],
                                 rhs=w1_t[:], start=True, stop=True)
                if t % 32 == 31 or t == NB - 1:
                    k = t // 32
                    w = (t % 32 + 1) * H
                    nc.scalar.copy(out=g_sb[:, k * 512:k * 512 + w],
                                   in_=psts[k][:, :w])
            nc.vector.tensor_tensor(out=g_sb[:], in0=g_sb[:], in1=disrep_t[:],
                                    op=mybir.AluOpType.mult)
            nc.sync.dma_start(out=g1[:], in_=g_sb[:])
    nc.compile()
    return nc


def _tree_reduce(nc, st, res_t, segs, d, clen_max):
    """In-place halving tree over each [cap, nb*d] slot-major segment;
    final level writes straight into res_t."""
    for (cap, b0, nb, soff) in segs:
        M = nb * d
        c = cap
        while c > 2:
            h = c // 2
            nc.vector.tensor_tensor(
                out=st[:, soff:soff + h * M],
                in0=st[:, soff:soff + h * M],
                in1=st[:, soff + (c - h) * M:soff + c * M],
                op=mybir.AluOpType.add)
            c -= h
        nc.vector.tensor_tensor(
            out=res_t[:, b0 * d:(b0 + nb) * d],
            in0=st[:, soff:soff + M],
            in1=st[:, soff + M:soff + 2 * M],
            op=mybir.AluOpType.add)


def _build_l2(plan_key, chunks, cols, has_b1):
    nc = bacc.Bacc("TRN2", target_bir_lowering=False, debug=False,
                   num_devices=NC)
    slots = nc.dram_tensor("slots", [128, cols], BF,
                           kind="ExternalInput").ap()
    disrep = nc.dram_tensor("disrep", [128, NB * H], BF,
                            kind="ExternalInput").ap()
    w2rep = nc.dram_tensor("w2rep", [128, NB * H], BF,
                           kind="ExternalInput").ap()
    dis1 = nc.dram_tensor("dis1", [128, NB], BF, kind="ExternalInput").ap()
    if has_b1:
        b1rep = nc.dram_tensor("b1rep", [128, NB * H], BF,
                               kind="ExternalInput").ap()
    g2 = nc.dram_tensor("g2", [128, NB], BF, kind="ExternalOutput").ap()
    chmax = max(cl for _, cl, _ in chunks)
    with tile.TileContext(nc) as tc:
        with tc.tile_pool(name="sb", bufs=min(3, len(chunks))) as sb, \
             tc.tile_pool(name="cst", bufs=1) as cst:
            res_t = cst.tile([128, NB * H], BF)
            disrep_t = cst.tile([128, NB * H], BF)
            w2rep_t = cst.tile([128, NB * H], BF)
            dis1_t = cst.tile([128, NB], BF)
            g2_t = cst.tile([128, NB], BF)
            if has_b1:
                b1rep_t = cst.tile([128, NB * H], BF)
            # big slot stream on the sync ring; consts on the scalar ring
            tiles = []
            for i, (coff, clen, segs) in enumerate(chunks):
                st = sb.tile([128, chmax], BF, tag="slotbuf")
                eng = nc.sync if i % 2 == 0 else nc.scalar
                eng.dma_start(out=st[:, :clen],
                              in_=slots[:, coff:coff + clen])
                tiles.append(st)
            nc.scalar.dma_start(out=disrep_t[:], in_=disrep[:])
            nc.scalar.dma_start(out=w2rep_t[:], in_=w2rep[:])
            nc.scalar.dma_start(out=dis1_t[:], in_=dis1[:])
            if has_b1:
                nc.scalar.dma_start(out=b1rep_t[:], in_=b1rep[:])
            for st, (coff, clen, segs) in zip(tiles, chunks):
                _tree_reduce(nc, st, res_t, segs, H, chmax)
            # epilogue: out1 = dis*agg (+b1); relu; *w2; sum_H; *dis
            nc.vector.tensor_tensor(out=res_t[:], in0=res_t[:],
                                    in1=disrep_t[:], op=mybir.AluOpType.mult)
            if has_b1:
                nc.vector.tensor_tensor(out=res_t[:], in0=res_t[:],
                                        in1=b1rep_t[:],
                                        op=mybir.AluOpType.add)
            nc.vector.scalar_tensor_tensor(
                out=res_t[:], in0=res_t[:], scalar=0.0, in1=w2rep_t[:],
                op0=mybir.AluOpType.max, op1=mybir.AluOpType.mult)
            r3 = res_t[:].rearrange("p (b c) -> p b c", b=NB, c=H)
            w = H
            while w > 2:
                h = w // 2
                nc.vector.tensor_tensor(out=r3[:, :, 0:h], in0=r3[:, :, 0:h],
                                        in1=r3[:, :, w - h:w],
                                        op=mybir.AluOpType.add)
                w -= h
            g2v = g2_t[:].rearrange("p (b o) -> p b o", b=NB, o=1)
            nc.vector.tensor_tensor(out=g2v, in0=r3[:, :, 0:1],
                                    in1=r3[:, :, 1:2],
                                    op=mybir.AluOpType.add)
            nc.vector.tensor_tensor(out=g2_t[:], in0=g2_t[:], in1=dis1_t[:],
                                    op=mybir.AluOpType.mult)
            nc.sync.dma_start(out=g2[:], in_=g2_t[:])
    nc.compile()
    return nc


def _build_l3(plan_key, chunks, cols, has_b2):
    nc = bacc.Bacc("TRN2", target_bir_lowering=False, debug=False,
                   num_devices=NC)
    slots = nc.dram_tensor("slots", [128, cols], BF,
                           kind="ExternalInput").ap()
    dis1 = nc.dram_tensor("dis1", [128, NB], BF, kind="ExternalInput").ap()
    b2c = nc.dram_tensor("b2c", [128, 1], FT, kind="ExternalInput").ap()
    out = nc.dram_tensor("out", [128, NB], FT, kind="ExternalOutput").ap()
    chmax = max(cl for _, cl, _ in chunks)
    with tile.TileContext(nc) as tc:
        with tc.tile_pool(name="sb", bufs=min(2, len(chunks))) as sb, \
             tc.tile_pool(name="cst", bufs=1) as cst:
            res_t = cst.tile([128, NB], BF)
            dis1_t = cst.tile([128, NB], BF)
            b2_t = cst.tile([128, 1], FT)
            out_t = cst.tile([128, NB], FT)
            tiles = []
            for i, (coff, clen, segs) in enumerate(chunks):
                st = sb.tile([128, chmax], BF, tag="slotbuf")
                eng = nc.sync if i % 2 == 0 else nc.scalar
                eng.dma_start(out=st[:, :clen],
                              in_=slots[:, coff:coff + clen])
                tiles.append(st)
            nc.scalar.dma_start(out=dis1_t[:], in_=dis1[:])
            nc.scalar.dma_start(out=b2_t[:], in_=b2c[:])
            for st, (coff, clen, segs) in zip(tiles, chunks):
                _tree_reduce(nc, st, res_t, segs, 1, chmax)
            nc.vector.tensor_tensor(out=out_t[:], in0=res_t[:],
                                    in1=dis1_t[:], op=mybir.AluOpType.mult)
            if has_b2:
                nc.vector.tensor_scalar(out=out_t[:], in0=out_t[:],
                                        scalar1=b2_t[:, 0:1], scalar2=None,
                                        op0=mybir.AluOpType.add)
            nc.sync.dma_start(out=out[:], in_=out_t[:])
    nc.compile()
    return nc


def _run(nc, in_maps, label):
    trace = os.environ.get("GCN_TRACE", "0") == "1"
    res = bass_utils.run_bass_kernel_spmd(nc, in_maps,
                                          core_ids=list(range(NC)),
                                          trace=trace)
    if res.exec_time_ns is not None:
        last_exec_ns[label] = res.exec_time_ns
    return res.results


# ---------------------------------------------------------------------
# host orchestration
# ---------------------------------------------------------------------
def kernel(x, edge_index, W1, b1, W2, b2):
    import ml_dtypes
    BFH = ml_dtypes.bfloat16
    x = np.asarray(x, np.float32)
    edge_index = np.asarray(edge_index, np.int32)
    W1 = np.asarray(W1, np.float32)
    b1 = np.asarray(b1, np.float32)
    W2 = np.asarray(W2, np.float32)
    b2 = np.asarray(b2, np.float32)

    # ---- host routing (integer index work) ----
    loop = np.arange(N, dtype=np.int64)
    src = np.concatenate([edge_index[0].astype(np.int64), loop])
    dst = np.concatenate([edge_index[1].astype(np.int64), loop])
    deg = np.bincount(dst, minlength=N).astype(np.int64)
    order = np.argsort(dst, kind="stable")
    src_s, dst_s = src[order], dst[order]
    core_start = np.searchsorted(dst_s, np.arange(0, N + 1, SH))

    # per-core degree-sorted row assignment + shared per-block slot caps
    pi = []           # pi[c][r] = global node id at row r (-1 = pad)
    caps_core = np.zeros((NC, NB), np.int64)
    for c in range(NC):
        d_loc = np.zeros(SHP, np.int64)
        d_loc[:SH] = deg[c * SH:(c + 1) * SH]
        ids = np.full(SHP, -1, np.int64)
        ids[:SH] = np.arange(c * SH, (c + 1) * SH)
        o = np.argsort(d_loc, kind="stable")
        pi.append(ids[o])
        caps_core[c] = np.maximum(
            CAP_R,
            ((d_loc[o].reshape(NB, 128).max(axis=1) + CAP_R - 1)
             // CAP_R) * CAP_R)
    caps = tuple(int(v) for v in caps_core.max(axis=0))
    cols16, chunks16, colbase16, stride16 = _plan(caps, H)
    cols1, chunks1, colbase1, stride1 = _plan(caps, 1)

    dis_full = np.where(deg > 0, 1.0 / np.sqrt(deg.astype(np.float64)),
                        0.0).astype(np.float32)
    has_b1 = bool(np.any(b1))
    has_b2 = bool(np.any(b2))

    # ---- L1: g1 = dis * (x @ W1) on device, node-partition layout ----
    l1 = _cached.get("l1") or _cached.setdefault("l1", _build_l1())
    in_maps1 = []
    for c in range(NC):
        xs = np.zeros((SHP, F), np.float32)
        xs[:SH] = x[c * SH:(c + 1) * SH]
        dis_sh = np.zeros(SHP, np.float32)
        dis_sh[:SH] = dis_full[c * SH:(c + 1) * SH]
        # disrep[p, t*16+ch] = dis[node t*128+p]
        disrep = np.repeat(dis_sh.reshape(NB, 128).T, H, axis=1)
        in_maps1.append({"xT": np.ascontiguousarray(xs.T).astype(BFH),
                         "w1": W1.astype(BFH),
                         "disrep": np.ascontiguousarray(disrep).astype(BFH)})
    res1 = _run(l1, in_maps1, "l1")
    g_bf = np.zeros((N, H), BFH)
    for c in range(NC):
        arr = np.asarray(res1[c]["g1"]).reshape(128, NB, H)
        g_bf[c * SH:(c + 1) * SH] = arr.transpose(1, 0, 2).reshape(
            SHP, H)[:SH]

    # ---- per-core slot coordinates (host, reused for L2/L3) ----
    coords = []       # (p_e, col16_e, col1_e, srcs_e)
    dis_pi16 = []     # disrep in pi order  [128, NB*H]
    dis_pi1 = []      # [128, NB]
    for c in range(NC):
        rows = pi[c]
        r = np.arange(SHP)
        valid = rows >= 0
        dis_r = np.where(valid, dis_full[np.where(valid, rows, 0)],
                         0.0).astype(np.float32)
        d1 = dis_r.reshape(NB, 128).T            # [128, NB]
        dis_pi1.append(np.ascontiguousarray(d1).astype(BFH))
        dis_pi16.append(np.ascontiguousarray(
            np.repeat(d1, H, axis=1)).astype(BFH))
        rr = r[valid]
        nodes_r = rows[valid]
        st = core_start[c] + np.searchsorted(
            dst_s[core_start[c]:core_start[c + 1]], nodes_r)
        cnt = deg[nodes_r]
        rep_r = np.repeat(rr, cnt)
        w_e = np.arange(len(rep_r)) - np.repeat(np.cumsum(cnt) - cnt, cnt)
        srcs_e = src_s[np.repeat(st, cnt) + w_e]
        b_e = rep_r // 128
        p_e = (rep_r % 128).astype(np.int32)
        col16_e = colbase16[b_e] + w_e * stride16[b_e]
        col1_e = colbase1[b_e] + w_e * stride1[b_e]
        coords.append((p_e, col16_e, col1_e, srcs_e))

    # ---- L2: conv1 reduce + relu + W2 on device ----
    key2 = ("l2", caps, has_b1)
    l2 = (_cached.get(key2)
          or _cached.setdefault(key2, _build_l2(caps, chunks16, cols16,
                                                has_b1)))
    w2rep = np.tile(W2[:, 0].astype(BFH)[None, :], (128, NB))
    b1rep = np.tile(b1.astype(BFH)[None, :], (128, NB))
    ch16 = np.arange(H, dtype=np.int64)
    in_maps2 = []
    for c in range(NC):
        p_e, col16_e, _, srcs_e = coords[c]
        sl = np.zeros((128, cols16), BFH)
        sl[p_e[:, None], col16_e[:, None] + ch16[None, :]] = g_bf[srcs_e]
        m = {"slots": sl, "disrep": dis_pi16[c], "w2rep": w2rep,
             "dis1": dis_pi1[c]}
        if has_b1:
            m["b1rep"] = b1rep
        in_maps2.append(m)
    res2 = _run(l2, in_maps2, "l2")
    g2_bf = np.zeros(N, BFH)
    for c in range(NC):
        g2c = np.asarray(res2[c]["g2"])          # [128, NB]
        rows = pi[c]
        r = np.arange(SHP)
        valid = rows >= 0
        g2_bf[rows[valid]] = g2c[(r % 128)[valid], (r // 128)[valid]]

    # ---- L3: conv2 reduce on device ----
    key3 = ("l3", caps, has_b2)
    l3 = (_cached.get(key3)
          or _cached.setdefault(key3, _build_l3(caps, chunks1, cols1,
                                                has_b2)))
    b2c = np.full((128, 1), float(b2[0]), np.float32)
    in_maps3 = []
    for c in range(NC):
        p_e, _, col1_e, srcs_e = coords[c]
        sl = np.zeros((128, cols1), BFH)
        sl[p_e, col1_e] = g2_bf[srcs_e]
        in_maps3.append({"slots": sl, "dis1": dis_pi1[c], "b2c": b2c})
    res3 = _run(l3, in_maps3, "l3")
    out = np.zeros((N, 1), np.float32)
    for c in range(NC):
        oc = np.asarray(res3[c]["out"])
        rows = pi[c]
        r = np.arange(SHP)
        valid = rows >= 0
        out[rows[valid], 0] = oc[(r % 128)[valid], (r // 128)[valid]]
    return out
